# revision 4
# baseline (speedup 1.0000x reference)
"""Trainium2 Bass kernel for nn_DecoderLayer (Informer decoder layer, ProbAttention).

Strategy: token-data-parallel across 8 NeuronCores for every dense GEMM
(QKV projections, attention out-projections, FFN) — these are ~95% of the
FLOPs (compute regime).  The irregular ProbAttention glue (35-sample score,
top-35 query selection, 35-row softmax, scatter) and the layernorms run on
host in fp32, replicating the reference math exactly.

Each GEMM: A [T=4096, K] @ W [K, N] -> [T, N], sharded by tokens: core c
gets A^T[:, 512c:512(c+1)] as lhsT and the full W as rhs, computing its 512
output rows.  fp32r matmul dtype (1 cycle/row on TRN2, ~1.6e-4 rel err,
measured on hw).
"""

import math

import numpy as np

import concourse.bacc as bacc
import concourse.mybir as mybir
from concourse import tile
from concourse.bass_utils import run_bass_kernel_spmd

N_CORES = 8
TOK = 4096          # B*L = 4*1024
TOK_SH = TOK // N_CORES
N_HEADS = 16
FACTOR = 5
EPS = 1e-5

_mm_cache: dict = {}
_last_hw_ns: list = []


def _build_mm(K: int, N: int):
    """Tiled matmul kernel: y[512, N] = a_shard^T.T @ w, fp32r."""
    nc = bacc.Bacc("TRN2", target_bir_lowering=False, debug=False,
                   num_devices=N_CORES)
    a = nc.declare_dram_parameter("a", [K, TOK_SH], mybir.dt.float32,
                                  isOutput=False)
    w = nc.declare_dram_parameter("w", [K, N], mybir.dt.float32,
                                  isOutput=False)
    y = nc.declare_dram_parameter("y", [TOK_SH, N], mybir.dt.float32,
                                  isOutput=True)
    KT = K // 128
    NG = max(1, N // 1024)      # outer n-groups of <=1024 (8 PSUM banks)
    NGW = min(N, 1024)          # columns per n-group
    with tile.TileContext(nc) as tc:
        with (
            tc.tile_pool(name="apool", bufs=1) as apool,
            tc.tile_pool(name="wpool", bufs=3) as wpool,
            tc.tile_pool(name="opool", bufs=3) as opool,
            tc.tile_pool(name="psum", bufs=1, space="PSUM") as psum,
        ):
            # resident lhsT: [128, KT*512] fp32r, a[kt*128+p, t] -> [p, kt*512+t]
            at = apool.tile([128, KT * TOK_SH], mybir.dt.float32r)
            for kt in range(KT):
                nc.gpsimd.dma_start(
                    at[:, kt * TOK_SH:(kt + 1) * TOK_SH],
                    a[kt * 128:(kt + 1) * 128, :])
            for ng in range(NG):
                accs = [psum.tile([128, 512], mybir.dt.float32,
                                  name=f"acc{i}", tag=f"acc{i}")
                        for i in range(4 * (NGW // 512))]
                for kt in range(KT):
                    wt = wpool.tile([128, NGW], mybir.dt.float32r)
                    nc.gpsimd.dma_start(
                        wt[:], w[kt * 128:(kt + 1) * 128,
                                 ng * NGW:(ng + 1) * NGW])
                    for tc4 in range(4):
                        lhs = at[:, kt * TOK_SH + tc4 * 128:
                                 kt * TOK_SH + (tc4 + 1) * 128]
                        for nch in range(NGW // 512):
                            nc.tensor.matmul(
                                accs[tc4 * (NGW // 512) + nch][:],
                                lhs,
                                wt[:, nch * 512:(nch + 1) * 512],
                                start=(kt == 0), stop=(kt == KT - 1))
                for tc4 in range(4):
                    for nch in range(NGW // 512):
                        ot = opool.tile([128, 512], mybir.dt.float32)
                        nc.scalar.copy(ot[:], accs[tc4 * (NGW // 512) + nch][:])
                        nc.sync.dma_start(
                            y[tc4 * 128:(tc4 + 1) * 128,
                              ng * NGW + nch * 512:ng * NGW + (nch + 1) * 512],
                            ot[:])
    nc.compile()
    return nc


def _dev_mm(A: np.ndarray, W: np.ndarray) -> np.ndarray:
    """A [4096, K] @ W [K, N] on 8 NeuronCores, token-sharded."""
    import time
    K, N = W.shape
    assert A.shape == (TOK, K)
    key = (K, N)
    if key not in _mm_cache:
        _mm_cache[key] = _build_mm(K, N)
    nc = _mm_cache[key]
    AT = np.ascontiguousarray(A.T.astype(np.float32))
    Wc = np.ascontiguousarray(W.astype(np.float32))
    in_maps = [{"a": np.ascontiguousarray(AT[:, c * TOK_SH:(c + 1) * TOK_SH]),
                "w": Wc} for c in range(N_CORES)]
    t0 = time.perf_counter()
    res = run_bass_kernel_spmd(nc, in_maps, list(range(N_CORES)))
    _last_hw_ns.append((time.perf_counter() - t0) * 1e9)
    return np.concatenate([res.results[c]["y"] for c in range(N_CORES)], axis=0)


def _layernorm(x, g, b):
    m = x.mean(-1, keepdims=True)
    v = x.var(-1, keepdims=True)
    return (x - m) / np.sqrt(v + EPS) * g + b


def _prob_attention(Q, K, V, mask_flag, idx):
    """Exact fp32 replication of the reference ProbAttention (numpy).

    Q/K/V: [B, L, H, D].  idx: [L_Q, U_part] precomputed jax sample indices.
    """
    Q = Q.transpose(0, 2, 1, 3)
    K = K.transpose(0, 2, 1, 3)
    V = V.transpose(0, 2, 1, 3)
    B, H, L_Q, D = Q.shape
    L_K = K.shape[2]
    u = min(FACTOR * int(np.ceil(np.log(L_Q))), L_Q)
    K_s = K[:, :, idx, :]                                # [B,H,L_Q,U,D]
    QKs = np.einsum('bhld,bhlsd->bhls', Q, K_s)
    M = QKs.max(-1) - QKs.sum(-1) / L_K                  # [B,H,L_Q]
    # top-u indices, matching jax.lax.top_k (stable, descending)
    M_top = np.argsort(-M, axis=-1, kind="stable")[..., :u]   # [B,H,u]
    bi = np.arange(B)[:, None, None]
    hi = np.arange(H)[None, :, None]
    Q_r = Q[bi, hi, M_top]                               # [B,H,u,D]
    scale = 1.0 / math.sqrt(D)
    scores = np.einsum('bhud,bhkd->bhuk', Q_r, K) * scale
    if mask_flag:
        causal = np.arange(L_K)[None, None, None, :] > M_top[..., None]
        scores = np.where(causal, -np.inf, scores)
        context = np.cumsum(V, axis=2)
    else:
        context = np.broadcast_to(V.mean(axis=2, keepdims=True),
                                  (B, H, L_Q, D)).copy()
    scores = scores - scores.max(-1, keepdims=True)
    e = np.exp(scores)
    attn = e / e.sum(-1, keepdims=True)
    ctx = np.einsum('bhuk,bhkd->bhud', attn, V)
    context[bi, hi, M_top] = ctx
    return context.transpose(0, 2, 1, 3)                 # [B,L,H,D]


def _sample_idx():
    """Replicate the reference's jax PRNG sampled key indices exactly."""
    import jax
    k1, k2 = jax.random.split(jax.random.key(42))
    L, U = 1024, min(FACTOR * int(np.ceil(np.log(1024))), 1024)
    i1 = np.asarray(jax.random.randint(k1, (L, U), 0, 1024))
    i2 = np.asarray(jax.random.randint(k2, (L, U), 0, 1024))
    return i1, i2


def kernel(x, enc_out, sa_Wq, sa_bq, sa_Wk, sa_bk, sa_Wv, sa_bv, sa_Wo, sa_bo,
           ca_Wq, ca_bq, ca_Wk, ca_bk, ca_Wv, ca_bv, ca_Wo, ca_bo,
           ln1_g, ln1_b, ln2_g, ln2_b, ln3_g, ln3_b,
           ff_W1, ff_b1, ff_W2, ff_b2):
    f32 = np.float32
    B, L, D = 4, 1024, 1024
    Dh = D // N_HEADS
    x = np.asarray(x, f32)
    enc = np.asarray(enc_out, f32)
    idx1, idx2 = _sample_idx()
    xf = x.reshape(TOK, D)
    encf = enc.reshape(TOK, D)

    # --- self attention -------------------------------------------------
    qkv_w = np.concatenate([sa_Wq, sa_Wk, sa_Wv], axis=1)     # [D, 3D]
    qkv = _dev_mm(xf, qkv_w) + np.concatenate([sa_bq, sa_bk, sa_bv])
    Q = qkv[:, :D].reshape(B, L, N_HEADS, Dh)
    Kt = qkv[:, D:2 * D].reshape(B, L, N_HEADS, Dh)
    Vt = qkv[:, 2 * D:].reshape(B, L, N_HEADS, Dh)
    ctx = _prob_attention(Q, Kt, Vt, True, idx1).reshape(TOK, D)
    o = _dev_mm(ctx, sa_Wo) + sa_bo
    x1 = _layernorm(xf + o, ln1_g, ln1_b).astype(f32)

    # --- cross attention ------------------------------------------------
    kv_w = np.concatenate([ca_Wk, ca_Wv], axis=1)             # [D, 2D]
    kv = _dev_mm(encf, kv_w) + np.concatenate([ca_bk, ca_bv])
    Qc = (_dev_mm(x1, ca_Wq) + ca_bq).reshape(B, L, N_HEADS, Dh)
    Kc = kv[:, :D].reshape(B, L, N_HEADS, Dh)
    Vc = kv[:, D:].reshape(B, L, N_HEADS, Dh)
    ctx2 = _prob_attention(Qc, Kc, Vc, False, idx2).reshape(TOK, D)
    o2 = _dev_mm(ctx2, ca_Wo) + ca_bo
    x2 = _layernorm(x1 + o2, ln2_g, ln2_b).astype(f32)

    # --- FFN -------------------------------------------------------------
    h = np.maximum(_dev_mm(x2, np.asarray(ff_W1, f32)) + ff_b1, 0.0)
    o3 = _dev_mm(h.astype(f32), np.asarray(ff_W2, f32)) + ff_b2
    out = _layernorm(x2 + o3, ln3_g, ln3_b).astype(f32)
    return out.reshape(B, L, D)


# revision 6
# speedup vs baseline: 1.0998x; 1.0998x over previous
"""Trainium2 Bass kernel for nn_DecoderLayer (Informer decoder layer, ProbAttention).

Strategy: token-data-parallel across 8 NeuronCores for every dense GEMM
(QKV projections, attention out-projections, FFN) — these are ~95% of the
FLOPs (compute regime).  The irregular ProbAttention glue (35-sample score,
top-35 query selection, 35-row softmax, scatter) and the layernorms run on
host in fp32, replicating the reference math exactly.

Each GEMM: A [T=4096, K] @ W [K, N] -> [T, N], sharded by tokens: core c
gets A^T[:, 512c:512(c+1)] as lhsT and the full W as rhs, computing its 512
output rows.  fp32r matmul dtype (1 cycle/row on TRN2, ~1.6e-4 rel err,
measured on hw).
"""

import math

import numpy as np

import concourse.bacc as bacc
import concourse.mybir as mybir
from concourse import tile
from concourse.bass_utils import run_bass_kernel_spmd

N_CORES = 8
TOK = 4096          # B*L = 4*1024
TOK_SH = TOK // N_CORES
N_HEADS = 16
FACTOR = 5
EPS = 1e-5

_mm_cache: dict = {}
_last_hw_ns: list = []


def _build_mm(K: int, N: int):
    """Tiled matmul kernel: y[512, N] = a_shard^T.T @ w, fp32r."""
    nc = bacc.Bacc("TRN2", target_bir_lowering=False, debug=False,
                   num_devices=N_CORES)
    a = nc.declare_dram_parameter("a", [K, TOK_SH], mybir.dt.float32,
                                  isOutput=False)
    w = nc.declare_dram_parameter("w", [K, N], mybir.dt.float32,
                                  isOutput=False)
    y = nc.declare_dram_parameter("y", [TOK_SH, N], mybir.dt.float32,
                                  isOutput=True)
    KT = K // 128
    NG = max(1, N // 1024)      # outer n-groups of <=1024 (8 PSUM banks)
    NGW = min(N, 1024)          # columns per n-group
    with tile.TileContext(nc) as tc:
        with (
            tc.tile_pool(name="apool", bufs=1) as apool,
            tc.tile_pool(name="wpool", bufs=3) as wpool,
            tc.tile_pool(name="opool", bufs=3) as opool,
            tc.tile_pool(name="psum", bufs=1, space="PSUM") as psum,
        ):
            # resident lhsT: [128, KT*512] fp32r, a[kt*128+p, t] -> [p, kt*512+t]
            at = apool.tile([128, KT * TOK_SH], mybir.dt.float32)
            for kt in range(KT):
                nc.sync.dma_start(
                    at[:, kt * TOK_SH:(kt + 1) * TOK_SH],
                    a[kt * 128:(kt + 1) * 128, :])
            for ng in range(NG):
                accs = [psum.tile([128, 512], mybir.dt.float32,
                                  name=f"acc{i}", tag=f"acc{i}")
                        for i in range(4 * (NGW // 512))]
                for kt in range(KT):
                    wt = wpool.tile([128, NGW], mybir.dt.float32)
                    nc.sync.dma_start(
                        wt[:], w[kt * 128:(kt + 1) * 128,
                                 ng * NGW:(ng + 1) * NGW])
                    for tc4 in range(4):
                        lhs = at[:, kt * TOK_SH + tc4 * 128:
                                 kt * TOK_SH + (tc4 + 1) * 128]
                        for nch in range(NGW // 512):
                            nc.tensor.matmul(
                                accs[tc4 * (NGW // 512) + nch][:],
                                lhs,
                                wt[:, nch * 512:(nch + 1) * 512],
                                start=(kt == 0), stop=(kt == KT - 1))
                for tc4 in range(4):
                    for nch in range(NGW // 512):
                        ot = opool.tile([128, 512], mybir.dt.float32)
                        nc.scalar.copy(ot[:], accs[tc4 * (NGW // 512) + nch][:])
                        nc.sync.dma_start(
                            y[tc4 * 128:(tc4 + 1) * 128,
                              ng * NGW + nch * 512:ng * NGW + (nch + 1) * 512],
                            ot[:])
    nc.compile()
    return nc


def _dev_mm(A: np.ndarray, W: np.ndarray) -> np.ndarray:
    """A [4096, K] @ W [K, N] on 8 NeuronCores, token-sharded."""
    import time
    K, N = W.shape
    assert A.shape == (TOK, K)
    key = (K, N)
    if key not in _mm_cache:
        _mm_cache[key] = _build_mm(K, N)
    nc = _mm_cache[key]
    AT = np.ascontiguousarray(A.T.astype(np.float32))
    Wc = np.ascontiguousarray(W.astype(np.float32))
    in_maps = [{"a": np.ascontiguousarray(AT[:, c * TOK_SH:(c + 1) * TOK_SH]),
                "w": Wc} for c in range(N_CORES)]
    t0 = time.perf_counter()
    res = run_bass_kernel_spmd(nc, in_maps, list(range(N_CORES)))
    _last_hw_ns.append((time.perf_counter() - t0) * 1e9)
    return np.concatenate([res.results[c]["y"] for c in range(N_CORES)], axis=0)


def _layernorm(x, g, b):
    m = x.mean(-1, keepdims=True)
    v = x.var(-1, keepdims=True)
    return (x - m) / np.sqrt(v + EPS) * g + b


def _prob_attention(Q, K, V, mask_flag, idx):
    """Exact fp32 replication of the reference ProbAttention (numpy).

    Q/K/V: [B, L, H, D].  idx: [L_Q, U_part] precomputed jax sample indices.
    """
    Q = Q.transpose(0, 2, 1, 3)
    K = K.transpose(0, 2, 1, 3)
    V = V.transpose(0, 2, 1, 3)
    B, H, L_Q, D = Q.shape
    L_K = K.shape[2]
    u = min(FACTOR * int(np.ceil(np.log(L_Q))), L_Q)
    K_s = K[:, :, idx, :]                                # [B,H,L_Q,U,D]
    QKs = np.einsum('bhld,bhlsd->bhls', Q, K_s)
    M = QKs.max(-1) - QKs.sum(-1) / L_K                  # [B,H,L_Q]
    # top-u indices, matching jax.lax.top_k (stable, descending)
    M_top = np.argsort(-M, axis=-1, kind="stable")[..., :u]   # [B,H,u]
    bi = np.arange(B)[:, None, None]
    hi = np.arange(H)[None, :, None]
    Q_r = Q[bi, hi, M_top]                               # [B,H,u,D]
    scale = 1.0 / math.sqrt(D)
    scores = np.einsum('bhud,bhkd->bhuk', Q_r, K) * scale
    if mask_flag:
        causal = np.arange(L_K)[None, None, None, :] > M_top[..., None]
        scores = np.where(causal, -np.inf, scores)
        context = np.cumsum(V, axis=2)
    else:
        context = np.broadcast_to(V.mean(axis=2, keepdims=True),
                                  (B, H, L_Q, D)).copy()
    scores = scores - scores.max(-1, keepdims=True)
    e = np.exp(scores)
    attn = e / e.sum(-1, keepdims=True)
    ctx = np.einsum('bhuk,bhkd->bhud', attn, V)
    context[bi, hi, M_top] = ctx
    return context.transpose(0, 2, 1, 3)                 # [B,L,H,D]


def _sample_idx():
    """Replicate the reference's jax PRNG sampled key indices exactly."""
    import jax
    k1, k2 = jax.random.split(jax.random.key(42))
    L, U = 1024, min(FACTOR * int(np.ceil(np.log(1024))), 1024)
    i1 = np.asarray(jax.random.randint(k1, (L, U), 0, 1024))
    i2 = np.asarray(jax.random.randint(k2, (L, U), 0, 1024))
    return i1, i2


def kernel(x, enc_out, sa_Wq, sa_bq, sa_Wk, sa_bk, sa_Wv, sa_bv, sa_Wo, sa_bo,
           ca_Wq, ca_bq, ca_Wk, ca_bk, ca_Wv, ca_bv, ca_Wo, ca_bo,
           ln1_g, ln1_b, ln2_g, ln2_b, ln3_g, ln3_b,
           ff_W1, ff_b1, ff_W2, ff_b2):
    f32 = np.float32
    B, L, D = 4, 1024, 1024
    Dh = D // N_HEADS
    x = np.asarray(x, f32)
    enc = np.asarray(enc_out, f32)
    idx1, idx2 = _sample_idx()
    xf = x.reshape(TOK, D)
    encf = enc.reshape(TOK, D)

    # --- self attention -------------------------------------------------
    qkv_w = np.concatenate([sa_Wq, sa_Wk, sa_Wv], axis=1)     # [D, 3D]
    qkv = _dev_mm(xf, qkv_w) + np.concatenate([sa_bq, sa_bk, sa_bv])
    Q = qkv[:, :D].reshape(B, L, N_HEADS, Dh)
    Kt = qkv[:, D:2 * D].reshape(B, L, N_HEADS, Dh)
    Vt = qkv[:, 2 * D:].reshape(B, L, N_HEADS, Dh)
    ctx = _prob_attention(Q, Kt, Vt, True, idx1).reshape(TOK, D)
    o = _dev_mm(ctx, sa_Wo) + sa_bo
    x1 = _layernorm(xf + o, ln1_g, ln1_b).astype(f32)

    # --- cross attention ------------------------------------------------
    kv_w = np.concatenate([ca_Wk, ca_Wv], axis=1)             # [D, 2D]
    kv = _dev_mm(encf, kv_w) + np.concatenate([ca_bk, ca_bv])
    Qc = (_dev_mm(x1, ca_Wq) + ca_bq).reshape(B, L, N_HEADS, Dh)
    Kc = kv[:, :D].reshape(B, L, N_HEADS, Dh)
    Vc = kv[:, D:].reshape(B, L, N_HEADS, Dh)
    ctx2 = _prob_attention(Qc, Kc, Vc, False, idx2).reshape(TOK, D)
    o2 = _dev_mm(ctx2, ca_Wo) + ca_bo
    x2 = _layernorm(x1 + o2, ln2_g, ln2_b).astype(f32)

    # --- FFN -------------------------------------------------------------
    h = np.maximum(_dev_mm(x2, np.asarray(ff_W1, f32)) + ff_b1, 0.0)
    o3 = _dev_mm(h.astype(f32), np.asarray(ff_W2, f32)) + ff_b2
    out = _layernorm(x2 + o3, ln3_g, ln3_b).astype(f32)
    return out.reshape(B, L, D)
